# revision 8
# baseline (speedup 1.0000x reference)
"""Attention-pooling kernel for 8 Trainium2 NeuronCores.

Reference computation (per batch b):
    h      = tanh(emb @ W + bias)          # [T, 512]
    s      = tanh(h @ c)                   # [T]
    a      = softmax(s)                    # scores in [-1, 1] -> no max-sub needed
    pooled = sum_t a_t * emb[t]            # [512]
    out    = relu(pooled @ w1 + b1) @ w2 + b2

Strategy: pure data-parallel over batch (4 per core).  Host pre-casts the
embedding to bf16 so the device can DMA-xbar-transpose it straight from HBM
into [d, t] layout (the matmul contraction needs d on partitions).  The
softmax needs no running max since scores are tanh-bounded, so a single
streaming pass accumulates exp-weighted sums via VectorE tensor_tensor_reduce.
"""

import os

import numpy as np
import ml_dtypes

B, T, D = 32, 4096, 512
H = 1024
DOUT = 8
NCORES = 8
BL = B // NCORES  # batches per core
P = 128
KD = D // P        # 4 contraction chunks for d=512
MO = D // P        # 4 output chunks for d_out=512
TT = 512           # t-tile (free dim per matmul / PSUM bank)
NTT = T // TT      # 8 t-tiles per batch

_last_results = None  # stashed BassKernelResults for test.py profiling


def _build_graph():
    from contextlib import ExitStack

    import concourse.bass as bass
    import concourse.mybir as mybir
    import concourse.tile as tile

    bf16 = mybir.dt.bfloat16
    f32 = mybir.dt.float32
    AF = mybir.ActivationFunctionType
    ALU = mybir.AluOpType

    from concourse.bacc import Bacc

    nc = Bacc(None, target_bir_lowering=False)

    emb = nc.declare_dram_parameter("emb", [BL, T, D], bf16, isOutput=False)
    w_d = nc.declare_dram_parameter("w_sb", [P, KD, D], bf16, isOutput=False)
    c_d = nc.declare_dram_parameter("c_sb", [P, KD], bf16, isOutput=False)
    b_d = nc.declare_dram_parameter("b_sb", [P, MO], f32, isOutput=False)
    w1_d = nc.declare_dram_parameter("w1_sb", [P, KD, H], bf16, isOutput=False)
    b1_d = nc.declare_dram_parameter("b1_sb", [P, H // P], f32, isOutput=False)
    w2_d = nc.declare_dram_parameter("w2_sb", [P, H // P, DOUT], bf16, isOutput=False)
    b2_d = nc.declare_dram_parameter("b2_sb", [DOUT, 1], f32, isOutput=False)
    out_d = nc.declare_dram_parameter("out", [DOUT, BL], f32, isOutput=True)

    with tile.TileContext(nc) as tc, ExitStack() as ctx:
        const = ctx.enter_context(tc.tile_pool(name="const", bufs=1))
        embp = ctx.enter_context(tc.tile_pool(name="embp", bufs=2))
        hp = ctx.enter_context(tc.tile_pool(name="hp", bufs=3))
        wp = ctx.enter_context(tc.tile_pool(name="wp", bufs=3))
        scr = ctx.enter_context(tc.tile_pool(name="scr", bufs=2))
        clsp = ctx.enter_context(tc.tile_pool(name="clsp", bufs=1))
        zpsum = ctx.enter_context(tc.tile_pool(name="zpsum", bufs=3, space="PSUM"))
        spsum = ctx.enter_context(tc.tile_pool(name="spsum", bufs=2, space="PSUM"))
        cpsum = ctx.enter_context(tc.tile_pool(name="cpsum", bufs=1, space="PSUM"))

        # --- params into SBUF (once) ---
        w_t = const.tile([P, KD, D], bf16)
        c_t = const.tile([P, KD], bf16)
        b_t = const.tile([P, MO], f32)
        w1_t = const.tile([P, KD, H], bf16)
        b1_t = const.tile([P, H // P], f32)
        w2_t = const.tile([P, H // P, DOUT], bf16)
        b2_t = const.tile([DOUT, 1], f32)
        for dst, src in (
            (w_t, w_d), (c_t, c_d), (b_t, b_d), (w1_t, w1_d),
            (b1_t, b1_d), (w2_t, w2_d), (b2_t, b2_d),
        ):
            nc.sync.dma_start(out=dst[:], in_=src[:])

        # accumulators (written as [P,1] slices, reduced at the end)
        pool_parts = const.tile([P, KD, BL, NTT], f32)
        denoms = const.tile([1, BL * NTT], f32)

        for b in range(BL):
            embT = embp.tile([P, KD, T], bf16)  # embT[p, j, t] = emb[b, t, j*128+p]
            for j in range(KD):
                nc.sync.dma_start(
                    out=embT[:, j],
                    in_=emb[b, :, j * P:(j + 1) * P],
                    transpose=True,
                )
            for ti in range(NTT):
                ts = slice(ti * TT, (ti + 1) * TT)
                hT = hp.tile([P, MO, TT], bf16)
                for mo in range(MO):
                    zps = zpsum.tile([P, TT], f32)
                    for ki in range(KD):
                        nc.tensor.matmul(
                            zps[:],
                            w_t[:, ki, mo * P:(mo + 1) * P],
                            embT[:, ki, ts],
                            start=(ki == 0),
                            stop=(ki == KD - 1),
                        )
                    nc.scalar.activation(
                        hT[:, mo], zps[:], AF.Tanh, bias=b_t[:, mo:mo + 1]
                    )
                # scores row: s[0, t] = sum_d c_d * hT[d, t]
                sps = spsum.tile([1, TT], f32)
                for j in range(KD):
                    nc.tensor.matmul(
                        sps[:],
                        c_t[:, j:j + 1],
                        hT[:, j],
                        start=(j == 0),
                        stop=(j == KD - 1),
                    )
                st = wp.tile([1, TT], f32)
                nc.scalar.activation(st[:], sps[:], AF.Tanh)
                wrow = wp.tile([1, TT], bf16)
                idx = b * NTT + ti
                nc.scalar.activation(
                    wrow[:], st[:], AF.Exp,
                    accum_out=denoms[:1, idx:idx + 1],
                )
                wrep = wp.tile([P, TT], bf16)
                nc.gpsimd.partition_broadcast(wrep[:], wrow[:])
                for j in range(KD):
                    y = scr.tile([P, TT], bf16)
                    nc.vector.tensor_mul(y[:], embT[:, j, ts], wrep[:])
                    if j == 0:
                        # ScalarE path: Copy activation with fused free-axis sum
                        ysink = scr.tile([P, TT], bf16)
                        nc.scalar.activation(
                            ysink[:], y[:], AF.Copy,
                            accum_out=pool_parts[:, j, b, ti:ti + 1],
                        )
                    else:
                        nc.vector.tensor_reduce(
                            pool_parts[:, j, b, ti:ti + 1], y[:],
                            axis=mybir.AxisListType.X, op=ALU.add,
                        )

        # --- finalize: pooled = num / denom ---
        pooledn = clsp.tile([P, KD, BL], f32)
        for j in range(KD):
            for b in range(BL):
                nc.vector.tensor_reduce(
                    pooledn[:, j, b:b + 1], pool_parts[:, j, b, :],
                    axis=mybir.AxisListType.X, op=ALU.add,
                )
        dsum = clsp.tile([1, BL], f32)
        for b in range(BL):
            nc.vector.tensor_reduce(
                dsum[:1, b:b + 1], denoms[:1, b * NTT:(b + 1) * NTT],
                axis=mybir.AxisListType.X, op=ALU.add,
            )
        rden = clsp.tile([1, BL], f32)
        nc.vector.reciprocal(rden[:1], dsum[:1])
        rrep = clsp.tile([P, BL], f32)
        nc.gpsimd.partition_broadcast(rrep[:], rden[:1])
        pooled_bf = clsp.tile([P, KD, BL], bf16)
        for b in range(BL):
            nc.vector.tensor_scalar_mul(
                pooled_bf[:, :, b], pooledn[:, :, b], rrep[:, b:b + 1]
            )

        # --- classifier ---
        r1 = clsp.tile([P, H // P, BL], bf16)
        for mo in range(H // P):
            c1ps = cpsum.tile([P, BL], f32)
            for ki in range(KD):
                nc.tensor.matmul(
                    c1ps[:],
                    w1_t[:, ki, mo * P:(mo + 1) * P],
                    pooled_bf[:, ki, :],
                    start=(ki == 0),
                    stop=(ki == KD - 1),
                )
            nc.scalar.activation(
                r1[:, mo], c1ps[:], AF.Relu, bias=b1_t[:, mo:mo + 1]
            )
        ops = cpsum.tile([DOUT, BL], f32)
        for ko in range(H // P):
            nc.tensor.matmul(
                ops[:],
                w2_t[:, ko, :],
                r1[:, ko, :],
                start=(ko == 0),
                stop=(ko == H // P - 1),
            )
        outsb = clsp.tile([DOUT, BL], f32)
        nc.vector.tensor_scalar_add(outsb[:], ops[:], b2_t[:, :1])
        nc.sync.dma_start(out=out_d[:], in_=outsb[:])

    return nc


def kernel(**inputs) -> np.ndarray:
    global _last_results
    from concourse.bass_utils import run_bass_kernel_spmd

    emb = np.asarray(inputs["embedding"], dtype=np.float32)
    W = np.asarray(inputs["weight"], dtype=np.float32)
    bias = np.asarray(inputs["bias"], dtype=np.float32)
    c = np.asarray(inputs["context_weight"], dtype=np.float32)
    w1 = np.asarray(inputs["w1"], dtype=np.float32)
    b1 = np.asarray(inputs["b1"], dtype=np.float32)
    w2 = np.asarray(inputs["w2"], dtype=np.float32)
    b2 = np.asarray(inputs["b2"], dtype=np.float32)

    bf = ml_dtypes.bfloat16
    emb_bf = emb.astype(bf)
    w_sb = np.ascontiguousarray(
        W.reshape(KD, P, D).transpose(1, 0, 2)).astype(bf)
    c_sb = np.ascontiguousarray(c.reshape(KD, P).T).astype(bf)
    b_sb = np.ascontiguousarray(bias.reshape(MO, P).T).astype(np.float32)
    w1_sb = np.ascontiguousarray(
        w1.reshape(KD, P, H).transpose(1, 0, 2)).astype(bf)
    b1_sb = np.ascontiguousarray(b1.reshape(H // P, P).T).astype(np.float32)
    w2_sb = np.ascontiguousarray(
        w2.reshape(H // P, P, DOUT).transpose(1, 0, 2)).astype(bf)
    b2_sb = b2.reshape(DOUT, 1).astype(np.float32)

    nc = _build_graph()
    if not nc.is_finalized():
        nc.finalize()
    in_maps = []
    for i in range(NCORES):
        in_maps.append({
            "emb": np.ascontiguousarray(emb_bf[i * BL:(i + 1) * BL]),
            "w_sb": w_sb, "c_sb": c_sb, "b_sb": b_sb,
            "w1_sb": w1_sb, "b1_sb": b1_sb, "w2_sb": w2_sb, "b2_sb": b2_sb,
        })
    res = run_bass_kernel_spmd(
        nc, in_maps, core_ids=list(range(NCORES)),
        trace=bool(int(os.environ.get("KERNEL_TRACE", "0"))),
    )
    _last_results = res
    out = np.concatenate(
        [np.asarray(res.results[i]["out"], np.float32).T for i in range(NCORES)],
        axis=0,
    )
    return out


# revision 9
# speedup vs baseline: 1.0638x; 1.0638x over previous
"""Attention-pooling kernel for 8 Trainium2 NeuronCores.

Reference computation (per batch b):
    h      = tanh(emb @ W + bias)          # [T, 512]
    s      = tanh(h @ c)                   # [T]
    a      = softmax(s)                    # scores in [-1, 1] -> no max-sub needed
    pooled = sum_t a_t * emb[t]            # [512]
    out    = relu(pooled @ w1 + b1) @ w2 + b2

Strategy: pure data-parallel over batch (4 per core).  Host pre-casts the
embedding to bf16 so the device can DMA-xbar-transpose it straight from HBM
into [d, t] layout (the matmul contraction needs d on partitions).  The
softmax needs no running max since scores are tanh-bounded, so a single
streaming pass accumulates exp-weighted sums.
"""

import os

import numpy as np
import ml_dtypes

B, T, D = 32, 4096, 512
H = 1024
DOUT = 8
NCORES = 8
BL = B // NCORES   # batches per core
P = 128
KD = D // P        # 4 contraction chunks for d=512
MO = D // P        # 4 output chunks for d_out=512
FT = 1024          # free-dim compute tile (t)
NFT = T // FT      # 4 compute tiles per batch
NH = FT // 512     # 512-wide matmul slices per compute tile

_last_results = None  # stashed BassKernelResults for test.py profiling


def _build_graph():
    from contextlib import ExitStack

    import concourse.bass as bass
    import concourse.mybir as mybir
    import concourse.tile as tile
    from concourse.bacc import Bacc

    bf16 = mybir.dt.bfloat16
    f32 = mybir.dt.float32
    AF = mybir.ActivationFunctionType
    ALU = mybir.AluOpType

    nc = Bacc(None, target_bir_lowering=False)

    emb = nc.declare_dram_parameter("emb", [BL, T, D], bf16, isOutput=False)
    w_d = nc.declare_dram_parameter("w_sb", [P, KD, D], bf16, isOutput=False)
    c_d = nc.declare_dram_parameter("c_sb", [P, KD], bf16, isOutput=False)
    b_d = nc.declare_dram_parameter("b_sb", [P, MO], f32, isOutput=False)
    w1_d = nc.declare_dram_parameter("w1_sb", [P, KD, H], bf16, isOutput=False)
    b1_d = nc.declare_dram_parameter("b1_sb", [P, H // P], f32, isOutput=False)
    w2_d = nc.declare_dram_parameter("w2_sb", [P, H // P, DOUT], bf16, isOutput=False)
    b2_d = nc.declare_dram_parameter("b2_sb", [DOUT, 1], f32, isOutput=False)
    out_d = nc.declare_dram_parameter("out", [DOUT, BL], f32, isOutput=True)

    with tile.TileContext(nc) as tc, ExitStack() as ctx:
        const = ctx.enter_context(tc.tile_pool(name="const", bufs=1))
        embp = ctx.enter_context(tc.tile_pool(name="embp", bufs=2))
        hp = ctx.enter_context(tc.tile_pool(name="hp", bufs=3))
        wp = ctx.enter_context(tc.tile_pool(name="wp", bufs=4))
        scr = ctx.enter_context(tc.tile_pool(name="scr", bufs=4))
        clsp = ctx.enter_context(tc.tile_pool(name="clsp", bufs=1))
        zpsum = ctx.enter_context(tc.tile_pool(name="zpsum", bufs=2, space="PSUM"))
        spsum = ctx.enter_context(tc.tile_pool(name="spsum", bufs=2, space="PSUM"))
        cpsum = ctx.enter_context(tc.tile_pool(name="cpsum", bufs=1, space="PSUM"))

        # --- params into SBUF (once) ---
        w_t = const.tile([P, KD, D], bf16)
        c_t = const.tile([P, KD], bf16)
        b_t = const.tile([P, MO], f32)
        w1_t = const.tile([P, KD, H], bf16)
        b1_t = const.tile([P, H // P], f32)
        w2_t = const.tile([P, H // P, DOUT], bf16)
        b2_t = const.tile([DOUT, 1], f32)
        for dst, src in (
            (w_t, w_d), (c_t, c_d), (b_t, b_d), (w1_t, w1_d),
            (b1_t, b1_d), (w2_t, w2_d), (b2_t, b2_d),
        ):
            nc.sync.dma_start(out=dst[:], in_=src[:])

        # accumulators (written as [P,1] slices, reduced at the end)
        pool_parts = const.tile([P, KD, BL, NFT], f32)
        denoms = const.tile([1, BL * NFT], f32)

        for b in range(BL):
            embT = embp.tile([P, KD, T], bf16)  # embT[p, j, t] = emb[b, t, j*128+p]
            for j in range(KD):
                nc.sync.dma_start(
                    out=embT[:, j],
                    in_=emb[b, :, j * P:(j + 1) * P],
                    transpose=True,
                )
            for ti in range(NFT):
                ts = slice(ti * FT, (ti + 1) * FT)
                hT = hp.tile([P, MO, FT], bf16)
                for mo in range(MO):
                    zps = zpsum.tile([P, FT], f32)
                    for nh in range(NH):
                        ns = slice(nh * 512, (nh + 1) * 512)
                        tsn = slice(ti * FT + nh * 512, ti * FT + (nh + 1) * 512)
                        for ki in range(KD):
                            nc.tensor.matmul(
                                zps[:, ns],
                                w_t[:, ki, mo * P:(mo + 1) * P],
                                embT[:, ki, tsn],
                                start=(ki == 0),
                                stop=(ki == KD - 1),
                            )
                    nc.scalar.activation(
                        hT[:, mo], zps[:], AF.Tanh, bias=b_t[:, mo:mo + 1]
                    )
                # scores row: s[0, t] = sum_d c_d * hT[d, t]
                st = wp.tile([1, FT], f32)
                for nh in range(NH):
                    ns = slice(nh * 512, (nh + 1) * 512)
                    sps = spsum.tile([1, 512], f32)
                    for mo in range(MO):
                        nc.tensor.matmul(
                            sps[:],
                            c_t[:, mo:mo + 1],
                            hT[:, mo, ns],
                            start=(mo == 0),
                            stop=(mo == MO - 1),
                        )
                    nc.scalar.activation(st[:, ns], sps[:], AF.Tanh)
                wrow = wp.tile([1, FT], bf16)
                idx = b * NFT + ti
                nc.scalar.activation(
                    wrow[:], st[:], AF.Exp,
                    accum_out=denoms[:1, idx:idx + 1],
                )
                wrep = wp.tile([P, FT], bf16)
                nc.gpsimd.partition_broadcast(wrep[:], wrow[:])
                for j in range(KD):
                    y = scr.tile([P, FT], bf16)
                    nc.vector.tensor_mul(y[:], embT[:, j, ts], wrep[:])
                    nc.vector.tensor_reduce(
                        pool_parts[:, j, b, ti:ti + 1], y[:],
                        axis=mybir.AxisListType.X, op=ALU.add,
                    )

        # --- finalize: pooled = num / denom ---
        pooledn = clsp.tile([P, KD, BL], f32)
        for j in range(KD):
            for b in range(BL):
                nc.vector.tensor_reduce(
                    pooledn[:, j, b:b + 1], pool_parts[:, j, b, :],
                    axis=mybir.AxisListType.X, op=ALU.add,
                )
        dsum = clsp.tile([1, BL], f32)
        for b in range(BL):
            nc.vector.tensor_reduce(
                dsum[:1, b:b + 1], denoms[:1, b * NFT:(b + 1) * NFT],
                axis=mybir.AxisListType.X, op=ALU.add,
            )
        rden = clsp.tile([1, BL], f32)
        nc.vector.reciprocal(rden[:1], dsum[:1])
        rrep = clsp.tile([P, BL], f32)
        nc.gpsimd.partition_broadcast(rrep[:], rden[:1])
        pooled_bf = clsp.tile([P, KD, BL], bf16)
        for b in range(BL):
            nc.vector.tensor_scalar_mul(
                pooled_bf[:, :, b], pooledn[:, :, b], rrep[:, b:b + 1]
            )

        # --- classifier ---
        r1 = clsp.tile([P, H // P, BL], bf16)
        for mo in range(H // P):
            c1ps = cpsum.tile([P, BL], f32)
            for ki in range(KD):
                nc.tensor.matmul(
                    c1ps[:],
                    w1_t[:, ki, mo * P:(mo + 1) * P],
                    pooled_bf[:, ki, :],
                    start=(ki == 0),
                    stop=(ki == KD - 1),
                )
            nc.scalar.activation(
                r1[:, mo], c1ps[:], AF.Relu, bias=b1_t[:, mo:mo + 1]
            )
        ops = cpsum.tile([DOUT, BL], f32)
        for ko in range(H // P):
            nc.tensor.matmul(
                ops[:],
                w2_t[:, ko, :],
                r1[:, ko, :],
                start=(ko == 0),
                stop=(ko == H // P - 1),
            )
        outsb = clsp.tile([DOUT, BL], f32)
        nc.vector.tensor_scalar_add(outsb[:], ops[:], b2_t[:, :1])
        nc.sync.dma_start(out=out_d[:], in_=outsb[:])

    return nc


def kernel(**inputs) -> np.ndarray:
    global _last_results
    from concourse.bass_utils import run_bass_kernel_spmd

    emb = np.asarray(inputs["embedding"], dtype=np.float32)
    W = np.asarray(inputs["weight"], dtype=np.float32)
    bias = np.asarray(inputs["bias"], dtype=np.float32)
    c = np.asarray(inputs["context_weight"], dtype=np.float32)
    w1 = np.asarray(inputs["w1"], dtype=np.float32)
    b1 = np.asarray(inputs["b1"], dtype=np.float32)
    w2 = np.asarray(inputs["w2"], dtype=np.float32)
    b2 = np.asarray(inputs["b2"], dtype=np.float32)

    bf = ml_dtypes.bfloat16
    emb_bf = emb.astype(bf)
    w_sb = np.ascontiguousarray(
        W.reshape(KD, P, D).transpose(1, 0, 2)).astype(bf)
    c_sb = np.ascontiguousarray(c.reshape(KD, P).T).astype(bf)
    b_sb = np.ascontiguousarray(bias.reshape(MO, P).T).astype(np.float32)
    w1_sb = np.ascontiguousarray(
        w1.reshape(KD, P, H).transpose(1, 0, 2)).astype(bf)
    b1_sb = np.ascontiguousarray(b1.reshape(H // P, P).T).astype(np.float32)
    w2_sb = np.ascontiguousarray(
        w2.reshape(H // P, P, DOUT).transpose(1, 0, 2)).astype(bf)
    b2_sb = b2.reshape(DOUT, 1).astype(np.float32)

    nc = _build_graph()
    if not nc.is_finalized():
        nc.finalize()
    in_maps = []
    for i in range(NCORES):
        in_maps.append({
            "emb": np.ascontiguousarray(emb_bf[i * BL:(i + 1) * BL]),
            "w_sb": w_sb, "c_sb": c_sb, "b_sb": b_sb,
            "w1_sb": w1_sb, "b1_sb": b1_sb, "w2_sb": w2_sb, "b2_sb": b2_sb,
        })
    res = run_bass_kernel_spmd(
        nc, in_maps, core_ids=list(range(NCORES)),
        trace=bool(int(os.environ.get("KERNEL_TRACE", "0"))),
    )
    _last_results = res
    out = np.concatenate(
        [np.asarray(res.results[i]["out"], np.float32).T for i in range(NCORES)],
        axis=0,
    )
    return out


# revision 12
# speedup vs baseline: 1.0734x; 1.0091x over previous
"""Attention-pooling kernel for 8 Trainium2 NeuronCores.

Reference computation (per batch b):
    h      = tanh(emb @ W + bias)          # [T, 512]
    s      = tanh(h @ c)                   # [T]
    a      = softmax(s)                    # scores in [-1, 1] -> no max-sub needed
    pooled = sum_t a_t * emb[t]            # [512]
    out    = relu(pooled @ w1 + b1) @ w2 + b2

Strategy: pure data-parallel over batch (4 per core).  Host pre-casts the
embedding to bf16 so the device can DMA-xbar-transpose it straight from HBM
into [d, t] layout (the matmul contraction needs d on partitions).  The
softmax needs no running max since scores are tanh-bounded, so a single
streaming pass accumulates exp-weighted sums.
"""

import os

import numpy as np
import ml_dtypes

B, T, D = 32, 4096, 512
H = 1024
DOUT = 8
NCORES = 8
BL = B // NCORES   # batches per core
P = 128
KD = D // P        # 4 contraction chunks for d=512
MO = D // P        # 4 output chunks for d_out=512
FT = 1024          # free-dim compute tile (t)
NFT = T // FT      # 4 compute tiles per batch
NH = FT // 512     # 512-wide matmul slices per compute tile

_last_results = None  # stashed BassKernelResults for test.py profiling


def _build_graph():
    from contextlib import ExitStack

    import concourse.bass as bass
    import concourse.mybir as mybir
    import concourse.tile as tile
    from concourse.bacc import Bacc

    bf16 = mybir.dt.bfloat16
    f32 = mybir.dt.float32
    AF = mybir.ActivationFunctionType
    ALU = mybir.AluOpType

    nc = Bacc(None, target_bir_lowering=False)

    emb = nc.declare_dram_parameter("emb", [BL, T, D], bf16, isOutput=False)
    w_d = nc.declare_dram_parameter("w_sb", [P, KD, D], bf16, isOutput=False)
    c_d = nc.declare_dram_parameter("c_sb", [P, KD], bf16, isOutput=False)
    b_d = nc.declare_dram_parameter("b_sb", [P, MO], f32, isOutput=False)
    w1_d = nc.declare_dram_parameter("w1_sb", [P, KD, H], bf16, isOutput=False)
    b1_d = nc.declare_dram_parameter("b1_sb", [P, H // P], f32, isOutput=False)
    w2_d = nc.declare_dram_parameter("w2_sb", [P, H // P, DOUT], bf16, isOutput=False)
    b2_d = nc.declare_dram_parameter("b2_sb", [DOUT, 1], f32, isOutput=False)
    out_d = nc.declare_dram_parameter("out", [DOUT, BL], f32, isOutput=True)

    with tile.TileContext(nc) as tc, ExitStack() as ctx:
        const = ctx.enter_context(tc.tile_pool(name="const", bufs=1))
        embp = ctx.enter_context(tc.tile_pool(name="embp", bufs=2))
        hp = ctx.enter_context(tc.tile_pool(name="hp", bufs=3))
        wp = ctx.enter_context(tc.tile_pool(name="wp", bufs=4))
        scr = ctx.enter_context(tc.tile_pool(name="scr", bufs=4))
        clsp = ctx.enter_context(tc.tile_pool(name="clsp", bufs=1))
        zpsum = ctx.enter_context(tc.tile_pool(name="zpsum", bufs=2, space="PSUM"))
        spsum = ctx.enter_context(tc.tile_pool(name="spsum", bufs=2, space="PSUM"))
        cpsum = ctx.enter_context(tc.tile_pool(name="cpsum", bufs=1, space="PSUM"))

        # --- params into SBUF (once) ---
        w_t = const.tile([P, KD, D], bf16)
        c_t = const.tile([P, KD], bf16)
        b_t = const.tile([P, MO], f32)
        w1_t = const.tile([P, KD, H], bf16)
        b1_t = const.tile([P, H // P], f32)
        w2_t = const.tile([P, H // P, DOUT], bf16)
        b2_t = const.tile([DOUT, 1], f32)
        for dst, src in (
            (w_t, w_d), (c_t, c_d), (b_t, b_d), (w1_t, w1_d),
            (b1_t, b1_d), (w2_t, w2_d), (b2_t, b2_d),
        ):
            nc.sync.dma_start(out=dst[:], in_=src[:])

        # accumulators (written as [P,1] slices, reduced at the end)
        pool_parts = const.tile([P, KD, BL, NFT], f32)
        denoms = const.tile([1, BL * NFT], f32)

        for b in range(BL):
            embT = embp.tile([P, KD, T], bf16)  # embT[p, j, t] = emb[b, t, j*128+p]
            for j in range(KD):
                for tc_ in range(NFT):
                    tsl = slice(tc_ * FT, (tc_ + 1) * FT)
                    nc.sync.dma_start(
                        out=embT[:, j, tsl],
                        in_=emb[b, tsl, j * P:(j + 1) * P],
                        transpose=True,
                    )
            for ti in range(NFT):
                ts = slice(ti * FT, (ti + 1) * FT)
                hT = hp.tile([P, MO, FT], bf16)
                for mo in range(MO):
                    zps = zpsum.tile([P, FT], f32)
                    # ki outer / nh inner: each W chunk stays stationary for
                    # both 512-halves (halves LDWEIGHTS traffic)
                    for ki in range(KD):
                        for nh in range(NH):
                            ns = slice(nh * 512, (nh + 1) * 512)
                            tsn = slice(ti * FT + nh * 512,
                                        ti * FT + (nh + 1) * 512)
                            nc.tensor.matmul(
                                zps[:, ns],
                                w_t[:, ki, mo * P:(mo + 1) * P],
                                embT[:, ki, tsn],
                                start=(ki == 0),
                                stop=(ki == KD - 1),
                            )
                    nc.scalar.activation(
                        hT[:, mo], zps[:], AF.Tanh, bias=b_t[:, mo:mo + 1]
                    )
                # scores row: s[0, t] = sum_d c_d * hT[d, t]
                st = wp.tile([1, FT], f32)
                for nh in range(NH):
                    ns = slice(nh * 512, (nh + 1) * 512)
                    sps = spsum.tile([1, 512], f32)
                    for mo in range(MO):
                        nc.tensor.matmul(
                            sps[:],
                            c_t[:, mo:mo + 1],
                            hT[:, mo, ns],
                            start=(mo == 0),
                            stop=(mo == MO - 1),
                        )
                    nc.scalar.activation(st[:, ns], sps[:], AF.Tanh)
                wrow = wp.tile([1, FT], bf16)
                idx = b * NFT + ti
                nc.scalar.activation(
                    wrow[:], st[:], AF.Exp,
                    accum_out=denoms[:1, idx:idx + 1],
                )
                wrep = wp.tile([P, FT], bf16)
                nc.gpsimd.partition_broadcast(wrep[:], wrow[:])
                for j in range(KD):
                    y = scr.tile([P, FT], bf16)
                    nc.vector.tensor_mul(y[:], embT[:, j, ts], wrep[:])
                    nc.vector.tensor_reduce(
                        pool_parts[:, j, b, ti:ti + 1], y[:],
                        axis=mybir.AxisListType.X, op=ALU.add,
                    )

        # --- finalize: pooled = num / denom ---
        pooledn = clsp.tile([P, KD, BL], f32)
        nc.vector.tensor_reduce(
            pooledn[:], pool_parts[:],
            axis=mybir.AxisListType.X, op=ALU.add,
        )
        dsum = clsp.tile([1, BL], f32)
        nc.vector.tensor_reduce(
            dsum[:1], denoms[:1].rearrange("a (b t) -> a b t", t=NFT),
            axis=mybir.AxisListType.X, op=ALU.add,
        )
        rden = clsp.tile([1, BL], f32)
        nc.vector.reciprocal(rden[:1], dsum[:1])
        rrep = clsp.tile([P, BL], f32)
        nc.gpsimd.partition_broadcast(rrep[:], rden[:1])
        pooled_bf = clsp.tile([P, KD, BL], bf16)
        for b in range(BL):
            nc.vector.tensor_scalar_mul(
                pooled_bf[:, :, b], pooledn[:, :, b], rrep[:, b:b + 1]
            )

        # --- classifier ---
        r1 = clsp.tile([P, H // P, BL], bf16)
        for mo in range(H // P):
            c1ps = cpsum.tile([P, BL], f32)
            for ki in range(KD):
                nc.tensor.matmul(
                    c1ps[:],
                    w1_t[:, ki, mo * P:(mo + 1) * P],
                    pooled_bf[:, ki, :],
                    start=(ki == 0),
                    stop=(ki == KD - 1),
                )
            nc.scalar.activation(
                r1[:, mo], c1ps[:], AF.Relu, bias=b1_t[:, mo:mo + 1]
            )
        ops = cpsum.tile([DOUT, BL], f32)
        for ko in range(H // P):
            nc.tensor.matmul(
                ops[:],
                w2_t[:, ko, :],
                r1[:, ko, :],
                start=(ko == 0),
                stop=(ko == H // P - 1),
            )
        outsb = clsp.tile([DOUT, BL], f32)
        nc.vector.tensor_scalar_add(outsb[:], ops[:], b2_t[:, :1])
        nc.sync.dma_start(out=out_d[:], in_=outsb[:])

    return nc


def kernel(**inputs) -> np.ndarray:
    global _last_results
    from concourse.bass_utils import run_bass_kernel_spmd

    emb = np.asarray(inputs["embedding"], dtype=np.float32)
    W = np.asarray(inputs["weight"], dtype=np.float32)
    bias = np.asarray(inputs["bias"], dtype=np.float32)
    c = np.asarray(inputs["context_weight"], dtype=np.float32)
    w1 = np.asarray(inputs["w1"], dtype=np.float32)
    b1 = np.asarray(inputs["b1"], dtype=np.float32)
    w2 = np.asarray(inputs["w2"], dtype=np.float32)
    b2 = np.asarray(inputs["b2"], dtype=np.float32)

    bf = ml_dtypes.bfloat16
    emb_bf = emb.astype(bf)
    w_sb = np.ascontiguousarray(
        W.reshape(KD, P, D).transpose(1, 0, 2)).astype(bf)
    c_sb = np.ascontiguousarray(c.reshape(KD, P).T).astype(bf)
    b_sb = np.ascontiguousarray(bias.reshape(MO, P).T).astype(np.float32)
    w1_sb = np.ascontiguousarray(
        w1.reshape(KD, P, H).transpose(1, 0, 2)).astype(bf)
    b1_sb = np.ascontiguousarray(b1.reshape(H // P, P).T).astype(np.float32)
    w2_sb = np.ascontiguousarray(
        w2.reshape(H // P, P, DOUT).transpose(1, 0, 2)).astype(bf)
    b2_sb = b2.reshape(DOUT, 1).astype(np.float32)

    nc = _build_graph()
    if not nc.is_finalized():
        nc.finalize()
    in_maps = []
    for i in range(NCORES):
        in_maps.append({
            "emb": np.ascontiguousarray(emb_bf[i * BL:(i + 1) * BL]),
            "w_sb": w_sb, "c_sb": c_sb, "b_sb": b_sb,
            "w1_sb": w1_sb, "b1_sb": b1_sb, "w2_sb": w2_sb, "b2_sb": b2_sb,
        })
    res = run_bass_kernel_spmd(
        nc, in_maps, core_ids=list(range(NCORES)),
        trace=bool(int(os.environ.get("KERNEL_TRACE", "0"))),
    )
    _last_results = res
    out = np.concatenate(
        [np.asarray(res.results[i]["out"], np.float32).T for i in range(NCORES)],
        axis=0,
    )
    return out
